# revision 1
# baseline (speedup 1.0000x reference)
"""DynamicMLP (3-layer spiking MLP) Trainium2 kernel.

Data-parallel over batch: 512 rows -> 8 NeuronCores x 64 (no collectives;
host shards inputs / gathers outputs).

Per-core design ("h on partitions, (t, b) on free"):
  - GEMM: X[h, (t,b)] accumulates over d in PSUM. Weights are host-packed
    per h-tile strip (one contiguous DMA run per partition).
    Layers 1-2 split W ~= hi + lo*2^-14 with both parts fp16 (subnormals
    flushed host-side): the lo pass multiplies 2^-14-scaled spikes, so the
    pair reconstructs fp32 weights to ~2^-23 while running both passes at
    the PE's full 1 cyc/row fp16 rate with FastWeightLoad.
  - Layer 3 (output) runs a SINGLE fp32r (tf32-like 1+8+11) pass: the
    moving spikes are 0/1 so products are exact, weights are pre-rounded
    host-side to fp32r's 11 explicit mantissa bits, and the output layer
    has no downstream spike cascade, so ~2^-13 weights keep the end-to-end
    rel err ~1e-2 (measured vs reference on CPU) while halving L3 PE time.
  - Spikes: L1 scan writes fp16 hi + 2^-14-scaled lo (lo copy on the idle
    Scalar engine); L2 scan writes ONE fp32r 0/1 tile consumed directly by
    the fp32r matmuls. X stays fp32 in separate tiles (4-t sub-tiles so the
    "x" ring slots stay small, which pays for a 3-deep spike-tile ring that
    keeps tile reuse off the critical path). Chunk-outer emission keeps the
    in-order PE stream from head-of-line blocking on scan results; the last
    layer's scan emission is interleaved between its GEMM chunks so the
    tail scan starts as soon as chunk 0's X lands.
  - Neuron recurrence per timestep: fused DVE ops (scalar_tensor_tensor /
    copy_predicated / tensor_tensor) + is_gt spike writes, carrying the
    u-state as U_t = u_t / 1.529^t so every step is a single fused op per
    state (all per-step scalars are compile-time constants); Square and the
    PSUM->SBUF bias-copies run on the Scalar engine; the output
    accumulation O += s runs on GpSimd.
"""
import sys
sys.path.insert(0, '/opt/trn_rl_repo')

from dataclasses import dataclass

import numpy as np

import concourse.bass as bass
from concourse import bacc
import concourse.mybir as mybir
from concourse.tile import TileContext

F32 = mybir.dt.float32
F32R = mybir.dt.float32r
F16 = mybir.dt.float16
U16 = mybir.dt.uint16
U32 = mybir.dt.uint32
ALU = mybir.AluOpType
ACTF = mybir.ActivationFunctionType

CDECAY = 0.5
VTH = 0.5
TH_R = 0.021
TH_S = 0.132
TH_U = 0.529
TH_V = -0.172
UDK = 1.0 + TH_U  # 1.529


@dataclass(frozen=True)
class Cfg:
    n_cores: int = 8
    B: int = 512          # total batch
    D: int = 2048
    H1: int = 2048
    H2: int = 2048
    Dout: int = 1024
    T: int = 16

    @property
    def b(self):          # batch rows per core
        return self.B // self.n_cores

    @property
    def layers(self):
        # (G = output h-tiles, Kt = contraction k-tiles)
        return [(self.H1 // 128, self.D // 128),
                (self.H2 // 128, self.H1 // 128),
                (self.Dout // 128, self.H2 // 128)]

    def n_passes(self, li):
        return 1 if li == 2 else 2

    @property
    def n_chunks(self):
        n = max(1, (self.T * self.b) // 512)
        assert self.T % n == 0
        return n

    @property
    def chunk_t(self):
        return self.T // self.n_chunks

    def chunk_list(self, li):
        """[(t0, tlen)] GEMM chunking for layer li. The last two layers'
        final chunk is split in half so both the serial tail scan and the
        wait on the previous layer's scan are shorter."""
        base = [(c * self.chunk_t, self.chunk_t) for c in range(self.n_chunks)]
        if li >= 1 and self.chunk_t >= 2:
            t0, L = base[-1]
            base = base[:-1] + [(t0, L // 2), (t0 + L // 2, L - L // 2)]
        return base


FP16_MIN_NORMAL = np.float32(6.104e-5)
LO_SCALE = 14  # lo pass carries (W - hi) * 2^14 against spikes * 2^-14


def split_fp16(w):
    """W ~= hi + lo * 2^-14, both fp16 with subnormals flushed on the host so
    the PE's subnormal behavior never matters. Residual ~2^-23 * |W|."""
    w = np.asarray(w, np.float32)
    hi = w.astype(np.float16)
    hi = np.where(np.abs(hi.astype(np.float32)) < FP16_MIN_NORMAL,
                  np.float16(0), hi)
    lo = ((w - hi.astype(np.float32)) * np.float32(2.0 ** LO_SCALE)) \
        .astype(np.float16)
    lo = np.where(np.abs(lo.astype(np.float32)) < FP16_MIN_NORMAL,
                  np.float16(0), lo)
    return hi, lo


def round_fp32r(w):
    """Round-to-nearest-even to fp32r (1+8+11 bits, value in top 20 bits)."""
    u = np.asarray(w, np.float32).view(np.uint32)
    r = ((u >> 12) & 1) + np.uint32(0x7FF)
    return ((u.astype(np.uint64) + r) & np.uint64(0xFFFFF000)) \
        .astype(np.uint32).view(np.float32)


def build_nc(cfg: Cfg):
    nc = bacc.Bacc()
    b, T = cfg.b, cfg.T
    n_chunks, chunk_t = cfg.n_chunks, cfg.chunk_t
    chunk_cols = chunk_t * b
    (G1, K1), (G2, K2), (G3, K3) = cfg.layers
    Gmax = max(G1, G2, G3)

    # host-packed so every DMA is one long contiguous run per partition:
    # S0[c] : [128p, K1*chunk_cols], W{i} : [G, passes, 128p, Kt*128]
    S0 = nc.dram_tensor("S0", [n_chunks, 2, 128, K1 * chunk_cols], F16,
                        kind="ExternalInput")
    Wd = []
    for i, (G, Kt) in enumerate(cfg.layers):
        npass = cfg.n_passes(i)
        wdt = F32R if i == 2 else F16
        Wd.append(nc.dram_tensor(f"W{i}", [G, npass, 128, Kt * 128], wdt,
                                 kind="ExternalInput"))
    BIAS = nc.dram_tensor("BIAS", [128, G1 + G2 + G3], F32, kind="ExternalInput")
    OUT = nc.dram_tensor("OUT", [128, G3 * b], F32, kind="ExternalOutput")

    with TileContext(nc) as tc:
        with tc.tile_pool(name="big", bufs=2) as big, \
             tc.tile_pool(name="wp", bufs=3) as wp, \
             tc.tile_pool(name="st", bufs=1) as st, \
             tc.tile_pool(name="sc", bufs=2) as sc, \
             tc.tile_pool(name="ps", bufs=6, space="PSUM") as ps:

            bias = st.tile([128, G1 + G2 + G3], F32, tag="bias")
            nc.sync.dma_start(out=bias, in_=BIAS[:])
            k021 = st.tile([128, Gmax * b], F32, tag="k021")
            nc.vector.memset(k021[:], TH_R)
            O = st.tile([128, G3 * b], F32, tag="O")
            nc.scalar.memzero(O[:])

            # prev_s = [(tile, kind, t0, tlen)] spike tiles of the previous
            # layer. Layer 0's come from DRAM via DMA (emitted lazily inside
            # layer-1's chunk loop); layer-1's are written by the scan's
            # is_gt in fp16 (hi 0/1, lo 0/2^-14 halves); layer-2's are a
            # single fp32r 0/1 half consumed by the fp32r matmuls.
            prev_s = [None] * n_chunks

            def stage_s0(c, nparts=4):
                n = K1 * chunk_cols
                s0t = big.tile([128, 2 * n], F16, tag="s", bufs=3)
                q = n // nparts

                def emit(parts):
                    for j in parts:
                        if j < nparts:   # hi half pieces
                            nc.sync.dma_start(
                                out=s0t[:, j * q:(j + 1) * q],
                                in_=S0[c, 0, :, j * q:(j + 1) * q])
                        else:            # lo half
                            nc.sync.dma_start(out=s0t[:, n:], in_=S0[c, 1])
                prev_s[c] = (s0t, "k-major", c * chunk_t, chunk_t)
                return emit

            beta = [float(np.float32(UDK) ** t) for t in range(T + 1)]
            bias_cols = [0, G1, G1 + G2]

            for li, (G, Kt) in enumerate(cfg.layers):
                F = G * b
                is_last = li == len(cfg.layers) - 1
                chunks = cfg.chunk_list(li)
                npass = cfg.n_passes(li)
                wdt = F32R if li == 2 else F16
                Wp = Wd[li]
                bias_col = bias_cols[li]

                def moving(k, t0, tlen, s, Kt=Kt):
                    # find the prev-layer tile containing [t0, t0+tlen)
                    for ent in prev_s:
                        if ent is None:
                            continue
                        tile_, kind, pt0, ptlen = ent
                        if pt0 <= t0 and t0 + tlen <= pt0 + ptlen:
                            break
                    else:
                        raise AssertionError((t0, tlen))
                    if kind == "k-major":           # [p, (s, k, t_in, b)]
                        v = tile_[:].rearrange(
                            "p (s k t b) -> p s k t b", s=2, k=Kt, t=ptlen)
                        return v[:, s, k, t0 - pt0:t0 - pt0 + tlen, :]
                    if kind == "scan":   # fp16 pair [p, (s, t_in, g, b)]
                        v = tile_[:].rearrange(
                            "p (s t g b) -> p s t g b", s=2, t=ptlen, g=Kt)
                        return v[:, s, t0 - pt0:t0 - pt0 + tlen, k, :]
                    # "scan1": fp32r single [p, (t_in, g, b)]
                    assert s == 0
                    v = tile_[:].rearrange(
                        "p (t g b) -> p t g b", t=ptlen, g=Kt)
                    return v[:, t0 - pt0:t0 - pt0 + tlen, k, :]

                # ---- GEMM: X[g, (t,b)] = sum_k W.T[k,g] @ spikes[k, (t,b)]
                # chunk-OUTER so the in-order PE stream never head-of-line
                # blocks on spikes the previous layer's scan hasn't produced
                # yet. Weight strips are re-loaded per chunk (extra W DMA,
                # hidden under the PE stream). X uses 4-t sub-tiles so the
                # "x" ring slots stay small.
                xparts = []     # [(t0, tlen, x tile)]

                def emit_gemm_chunk(ci, li=li, G=G, Kt=Kt, F=F, npass=npass,
                                    wdt=wdt, Wp=Wp, bias_col=bias_col,
                                    chunks=chunks, moving=moving,
                                    xparts=xparts):
                    t0, tlen = chunks[ci]
                    s0_rest = None
                    if li == 0:
                        if ci == 0:
                            # fine-grained: first eighth ahead of g0's strip
                            # pieces so the very first matmuls aren't queued
                            # behind the whole chunk transfer
                            em = stage_s0(ci, nparts=8)
                            em([0])
                            s0_rest = em
                        else:
                            stage_s0(ci)(range(5))
                    subs = ([(t0 + 4 * j, 4) for j in range(tlen // 4)]
                            if tlen > 4 else [(t0, tlen)])
                    xvs = []
                    for st0, slen in subs:
                        xt = big.tile([128, slen * F], F32, tag="x", bufs=3,
                                      name="xt")
                        xv = xt[:].rearrange("p (t g b) -> p t g b",
                                             t=slen, g=G)
                        xvs.append(xv)
                        xparts.append((st0, slen, xt))
                    for g in range(G):
                        wstrip = wp.tile([128, npass * Kt * 128], wdt, tag="w",
                                         name="wstrip")
                        wsv = wstrip[:].rearrange("p (s k m) -> p s k m",
                                                  s=npass, k=Kt)
                        if s0_rest is not None:
                            # first strip: hi quarters interleaved with the
                            # remaining S0 eighths so the first matmuls only
                            # wait on ~0.4MB; all hi pieces before any lo
                            qk = Kt * 128 // 4
                            for piece in range(4):
                                nc.sync.dma_start(
                                    out=wstrip[:, piece * qk:(piece + 1) * qk],
                                    in_=Wp[g, 0, :, piece * qk:(piece + 1) * qk])
                                if piece < 3:
                                    s0_rest([piece + 1])
                            s0_rest([4, 5, 6, 7])
                            nc.sync.dma_start(out=wstrip[:, Kt * 128:],
                                              in_=Wp[g, 1])
                            s0_rest([8])
                            s0_rest = None
                        else:
                            nc.sync.dma_start(out=wstrip[:, :Kt * 128],
                                              in_=Wp[g, 0])
                            for si in range(1, npass):
                                nc.sync.dma_start(
                                    out=wstrip[:, si * Kt * 128:
                                               (si + 1) * Kt * 128],
                                    in_=Wp[g, si])
                        # full-width matmuls (N up to 512) into one psum;
                        # the PSUM->X copies split into the 4-t x tiles
                        psum = ps.tile([128, tlen * b], F32, tag="p",
                                       name="psum")
                        n_mm = npass * Kt
                        i = 0
                        for s in range(npass):
                            for k in range(Kt):
                                nc.tensor.matmul(
                                    psum[:], wsv[:, s, k],
                                    moving(k, t0, tlen, s),
                                    start=(i == 0), stop=(i == n_mm - 1))
                                i += 1
                        # PSUM -> X slices, + per-h bias (ScalarE)
                        pv = psum[:].rearrange("p (t b) -> p t b", t=tlen)
                        for j, (st0, slen) in enumerate(subs):
                            nc.scalar.activation(
                                xvs[j][:, :, g, :],
                                pv[:, st0 - t0:st0 - t0 + slen],
                                ACTF.Identity,
                                bias=bias[:, bias_col + g:bias_col + g + 1],
                                scale=1.0)

                # ---- neuron scan over t
                # spike output format of THIS layer's scan:
                #   li==0 -> fp16 hi + ScalarE lo ("scan")
                #   li==1 -> single fp32r 0/1 ("scan1")
                #   li==2 -> local fp16 scratch (only O += s survives)
                sdt = F32R if li == 1 else F16
                halves = 2 if li == 0 else 1
                mask_dt = U32 if li == 1 else U16

                def sp_val(sp, li=li):
                    return sp.bitcast(F32) if li == 1 else sp

                c_t = st.tile([128, F], F32, tag="c")
                v_t = st.tile([128, F], F32, tag="v")
                U_t = st.tile([128, F], F32, tag="U")
                cur_s = []
                sprev = [None]            # AP of s_{t-1} (boxed)

                def emit_scan_step(t, li=li, F=F, is_last=is_last, sdt=sdt,
                                   halves=halves, mask_dt=mask_dt,
                                   sp_val=sp_val, c_t=c_t, v_t=v_t, U_t=U_t,
                                   cur_s=cur_s, sprev=sprev, xparts=xparts):
                    pt0, plen, xt = next(e for e in xparts
                                         if e[0] <= t < e[0] + e[1])
                    x = xt[:, (t - pt0) * F:(t - pt0 + 1) * F]
                    if is_last:
                        s_tile = sc.tile([128, F], F16, tag="stmp",
                                         name="s_tile")
                        s_out = s_tile[:]
                        s_lo = None
                        s_rd = s_tile[:]
                    else:
                        s_ci, s_tin = t // chunk_t, t % chunk_t
                        if s_tin == 0:
                            stile = big.tile([128, halves * chunk_t * F], sdt,
                                             tag="s", bufs=3, name="stile")
                            cur_s.append((stile, "scan" if li == 0 else "scan1",
                                          s_ci * chunk_t, chunk_t))
                        stile = cur_s[s_ci][0]
                        s_out = stile[:, s_tin * F:(s_tin + 1) * F]
                        s_lo = (stile[:, (chunk_t + s_tin) * F:
                                      (chunk_t + s_tin + 1) * F]
                                if halves == 2 else None)
                        s_rd = s_out
                    if t == 0:
                        nc.scalar.copy(c_t[:], x)
                        nc.scalar.copy(v_t[:], x)
                        nc.scalar.memzero(U_t[:])
                        nc.vector.tensor_scalar(
                            out=s_out, in0=x, scalar1=VTH, scalar2=None,
                            op0=ALU.is_gt)
                        if s_lo is not None:
                            nc.scalar.activation(
                                s_lo, s_out, ACTF.Identity,
                                scale=float(2.0 ** -LO_SCALE))
                    else:
                        sp = sprev[0]
                        # c = 0.5c + x (emitted first: independent of s_{t-1})
                        nc.vector.scalar_tensor_tensor(
                            out=c_t[:], in0=c_t[:], scalar=CDECAY, in1=x,
                            op0=ALU.mult, op1=ALU.add)
                        # U += (0.132/beta_{t-1}) * s_{t-1}
                        nc.vector.scalar_tensor_tensor(
                            out=U_t[:], in0=sp_val(sp),
                            scalar=TH_S / beta[t - 1],
                            in1=U_t[:], op0=ALU.mult, op1=ALU.add)
                        # v reset on spike (mask: nonzero spike bits)
                        nc.vector.copy_predicated(
                            out=v_t[:], mask=sp.bitcast(mask_dt),
                            data=k021[:, :F])
                        # w = c - beta_{t-1} * U   (= c - u_pre)
                        w = sc.tile([128, F], F32, tag="w", bufs=1)
                        nc.vector.scalar_tensor_tensor(
                            out=w[:], in0=U_t[:], scalar=-beta[t - 1],
                            in1=c_t[:], op0=ALU.mult, op1=ALU.add)
                        # sq = v^2 (ScalarE)
                        sq = sc.tile([128, F], F32, tag="sq", bufs=1)
                        nc.scalar.activation(sq[:], v_t[:], ACTF.Square)
                        if t < T - 1:   # U_t is dead after the last step
                            # U = (-0.172/beta_t) * v + U
                            nc.vector.scalar_tensor_tensor(
                                out=U_t[:], in0=v_t[:], scalar=TH_V / beta[t],
                                in1=U_t[:], op0=ALU.mult, op1=ALU.add)
                        # v = sq + w
                        nc.vector.tensor_tensor(
                            out=v_t[:], in0=sq[:], in1=w[:], op=ALU.add)
                        if is_last and t == T - 1:
                            # fused: O += (v > 0.5); the spike tensor itself
                            # is dead after the last step
                            nc.vector.scalar_tensor_tensor(
                                out=O[:], in0=v_t[:], scalar=VTH, in1=O[:],
                                op0=ALU.is_gt, op1=ALU.add)
                            sprev[0] = None
                            return
                        # s_t = v > 0.5 (plus 2^-14-scaled lo copy on ScalarE
                        # for li==0)
                        nc.vector.tensor_scalar(
                            out=s_out, in0=v_t[:], scalar1=VTH, scalar2=None,
                            op0=ALU.is_gt)
                        if s_lo is not None:
                            nc.scalar.activation(
                                s_lo, s_out, ACTF.Identity,
                                scale=float(2.0 ** -LO_SCALE))
                    if is_last:
                        nc.gpsimd.tensor_tensor(
                            out=O[:], in0=O[:], in1=s_out[:], op=ALU.add)
                    sprev[0] = s_rd

                # emission schedule: for the last layer, interleave the scan
                # between GEMM chunks so its Squares / t0-init don't queue
                # behind later chunks' PSUM->X copies in the ScalarE FIFO and
                # the tail scan starts as soon as chunk 0's X lands.
                if is_last:
                    plan = [(1, range(0, 8)), (2, range(8, 12)),
                            (3, range(12, 16))]
                else:
                    plan = [(len(chunks), range(T))]
                done = 0
                for n_now, t_range in plan:
                    while done < n_now:
                        emit_gemm_chunk(done)
                        done += 1
                    for t in t_range:
                        emit_scan_step(t)

                if not is_last:
                    prev_s[:] = cur_s

            # rate decode scale (1/T) is folded into the host-side unpack
            nc.sync.dma_start(out=OUT[:], in_=O[:])

    return nc


def pack_inputs(cfg: Cfg, in_pop_spikes, W1, b1, W2, b2, Wout, bout):
    """Host-side packing -> list of per-core input maps."""
    (G1, _), (G2, _), (G3, _) = cfg.layers
    b, T = cfg.b, cfg.T

    weights = {}
    for i, W in enumerate([W1, W2, Wout]):
        W = np.asarray(W, np.float32)          # [H, D]
        H, D = W.shape
        G, Kt = H // 128, D // 128
        # W[h, d] with h = g*128 + m, d = k*128 + p; lhsT tile (g,k) = [p, m]
        WT = W.T.reshape(Kt, 128, G, 128)      # [k, p, g, m]
        strips = np.ascontiguousarray(
            WT.transpose(2, 1, 0, 3)).reshape(G, 128, Kt * 128)  # [g, p, (k,m)]
        if cfg.n_passes(i) == 1:
            pk = round_fp32r(strips)[:, None]   # [G, 1, 128, Kt*128] f32
        else:
            hi, lo = split_fp16(strips)
            pk = np.empty((G, 2, 128, Kt * 128), np.float16)
            pk[:, 0] = hi
            pk[:, 1] = lo
        weights[f"W{i}"] = pk

    bias = np.zeros((128, G1 + G2 + G3), np.float32)
    col = 0
    for G, vec in [(G1, b1), (G2, b2), (G3, bout)]:
        bias[:, col:col + G] = np.asarray(vec, np.float32).reshape(G, 128).T
        col += G

    # spikes [B, D, T] -> per core packed [n_chunks, 2(hi/lo), 128p, (k,t_in,b)]
    sp = np.asarray(in_pop_spikes, np.float32)
    K1 = cfg.D // 128
    nch, cht = cfg.n_chunks, cfg.chunk_t
    in_maps = []
    for core in range(cfg.n_cores):
        shard = sp[core * b:(core + 1) * b]            # [b, D, T]
        # [d, t, b] -> [k, p, c, t_in, b] -> [c, p, k, t_in, b]
        s0 = shard.transpose(1, 2, 0).reshape(K1, 128, nch, cht, b)
        s0 = np.ascontiguousarray(s0.transpose(2, 1, 0, 3, 4)) \
            .reshape(nch, 128, K1 * cht * b)
        s0d = np.empty((nch, 2, 128, K1 * cht * b), np.float16)
        s0d[:, 0] = s0
        s0d[:, 1] = s0 * np.float32(2.0 ** -LO_SCALE)
        in_maps.append(dict(S0=s0d, BIAS=bias, **weights))
    return in_maps


def unpack_outputs(cfg: Cfg, results):
    """Per-core OUT [128, G3*b] -> full [B, Dout]."""
    (_, _), (_, _), (G3, _) = cfg.layers
    b = cfg.b
    out = np.empty((cfg.B, cfg.Dout), np.float32)
    for core, r in enumerate(results):
        o = r["OUT"].reshape(128, G3, b) * np.float32(1.0 / cfg.T)
        out[core * b:(core + 1) * b] = o.transpose(2, 1, 0).reshape(b, cfg.Dout)
    return out


_NC_CACHE = {}


def _get_nc(cfg: Cfg):
    if cfg not in _NC_CACHE:
        nc = build_nc(cfg)
        nc.finalize()
        _NC_CACHE[cfg] = nc
    return _NC_CACHE[cfg]


def run(in_pop_spikes, W1, b1, W2, b2, Wout, bout, trace=False, **spmd_kwargs):
    from concourse import bass_utils
    cfg = Cfg()
    nc = _get_nc(cfg)
    in_maps = pack_inputs(cfg, in_pop_spikes, W1, b1, W2, b2, Wout, bout)
    res = bass_utils.run_bass_kernel_spmd(
        nc, in_maps, core_ids=list(range(cfg.n_cores)), trace=trace,
        **spmd_kwargs)
    return unpack_outputs(cfg, res.results), res


def kernel(in_pop_spikes, W1, b1, W2, b2, Wout, bout,
           batch_size=None, update=None, re_calibration=None, **_):
    out, _res = run(in_pop_spikes, W1, b1, W2, b2, Wout, bout)
    return out

